# revision 2
# baseline (speedup 1.0000x reference)
import sys
sys.path.insert(0, "/opt/trn_rl_repo")
import numpy as np
import concourse.bass as bass
import concourse.bacc as bacc
import concourse.tile as tile
from concourse import mybir
from concourse.bass_utils import run_bass_kernel_spmd

F32 = mybir.dt.float32
F32R = mybir.dt.float32r
AF = mybir.ActivationFunctionType
OP = mybir.AluOpType

N, CIO, L = 16, 512, 384
DIMHEAD, NUMHEAD, MAXEMBED, DIMGROUP = 64, 8, 384, 8
CHID = DIMHEAD * NUMHEAD
NCORES = 8
BPC = N // NCORES      # batches per core
KC = CIO // 128        # contraction chunks
OC = CHID // 128       # output-channel chunks
JC = L // 128          # key-position chunks


def build_nc(iters=1):
    nc = bacc.Bacc("TRN2", target_bir_lowering=False, debug=False)

    def inp(name, shape, dt):
        return nc.dram_tensor(name, shape, dt, kind="ExternalInput").ap()

    x = inp("x", [BPC, CIO, L], F32R)
    xorg = inp("xorg", [BPC, CIO, L], F32)
    abspos = inp("abspos", [BPC, CIO, L], F32)
    maskT = inp("maskT", [BPC, 128, JC], F32)
    norm = inp("norm", [BPC, 1], F32)
    vres = inp("vres", [128, KC], F32)
    qko = inp("qko", [128, KC], F32)
    qkp = inp("qkp", [128, KC], F32)
    relpos = inp("relpos", [768], F32R)
    gate_wT = inp("gate_wT", [CIO, NUMHEAD], F32R)
    gate_b = inp("gate_b", [NUMHEAD, 1], F32)
    q_wT = inp("q_wT", [CIO, CHID], F32R)
    k_wT = inp("k_wT", [CIO, CHID], F32R)
    v_wT = inp("v_wT", [CIO, CHID], F32R)
    dense_wT = inp("dense_wT", [CHID, CIO], F32R)
    dense_b = inp("dense_b", [128, OC], F32)
    i128 = inp("i128", [128, 128], F32R)
    i8 = inp("i8", [NUMHEAD, NUMHEAD], F32R)
    sel8 = inp("sel8", [128, NUMHEAD * NUMHEAD], F32R)
    bsel = inp("bsel", [NUMHEAD, NUMHEAD * DIMHEAD], F32R)
    out = nc.dram_tensor("out", [BPC, CIO, L], F32, kind="ExternalOutput").ap()

    with tile.TileContext(nc) as tc:
        with tc.tile_pool(name="wts", bufs=1) as wp, \
             tc.tile_pool(name="a2", bufs=2) as a2, \
             tc.tile_pool(name="a1", bufs=1) as a1, \
             tc.tile_pool(name="pp", bufs=6) as ppool, \
             tc.tile_pool(name="ps_big", bufs=2, space="PSUM") as psb, \
             tc.tile_pool(name="ps_s", bufs=2, space="PSUM") as pss, \
             tc.tile_pool(name="ps_o", bufs=2, space="PSUM") as pso, \
             tc.tile_pool(name="ps_dn", bufs=1, space="PSUM") as psd, \
             tc.tile_pool(name="ps_rb", bufs=1, space="PSUM") as psr:

            # ---- persistent weights/constants ----
            def wtile(src, shape, dt, tag):
                t = wp.tile(shape, dt, tag=tag)
                nc.sync.dma_start(out=t, in_=src)
                return t

            wq = [wtile(q_wT[kc * 128:(kc + 1) * 128, :], [128, CHID], F32R, f"wq{kc}") for kc in range(KC)]
            wk = [wtile(k_wT[kc * 128:(kc + 1) * 128, :], [128, CHID], F32R, f"wk{kc}") for kc in range(KC)]
            wv = [wtile(v_wT[kc * 128:(kc + 1) * 128, :], [128, CHID], F32R, f"wv{kc}") for kc in range(KC)]
            wd = [wtile(dense_wT[kc * 128:(kc + 1) * 128, :], [128, CIO], F32R, f"wd{kc}") for kc in range(KC)]
            wg = [wtile(gate_wT[kc * 128:(kc + 1) * 128, :], [128, NUMHEAD], F32R, f"wg{kc}") for kc in range(KC)]
            ti128 = wtile(i128, [128, 128], F32R, "i128")
            ti8 = wtile(i8, [NUMHEAD, NUMHEAD], F32R, "i8")
            tsel8 = wtile(sel8, [128, NUMHEAD * NUMHEAD], F32R, "sel8")
            tbsel = wtile(bsel, [NUMHEAD, NUMHEAD * DIMHEAD], F32R, "bsel")
            tvres = wtile(vres, [128, KC], F32, "vres")
            tqko = wtile(qko, [128, KC], F32, "qko")
            tqkp = wtile(qkp, [128, KC], F32, "qkp")
            tgb = wtile(gate_b, [NUMHEAD, 1], F32, "gb")
            tdb = wtile(dense_b, [128, OC], F32, "db")
            # Toeplitz relpos^T chunks: rpt[jc][p, i] = relpos[384 + 128*jc + p - i]
            trpt = []
            for jc in range(JC):
                t = wp.tile([128, L], F32R, tag=f"rpt{jc}")
                src = bass.AP(tensor=relpos.tensor, offset=MAXEMBED + 128 * jc,
                              ap=[[1, 128], [-1, L]])
                nc.sync.dma_start(out=t, in_=src)
                trpt.append(t)

            for _ in range(iters):
                for b in range(BPC):
                    # ---- load activations ----
                    tx, txo, tab = [], [], []
                    for kc in range(KC):
                        t = a2.tile([128, L], F32R, tag=f"x{kc}")
                        nc.sync.dma_start(out=t, in_=x[b, kc * 128:(kc + 1) * 128, :])
                        tx.append(t)
                        t = a2.tile([128, L], F32, tag=f"xo{kc}")
                        nc.sync.dma_start(out=t, in_=xorg[b, kc * 128:(kc + 1) * 128, :])
                        txo.append(t)
                        t = a2.tile([128, L], F32, tag=f"ab{kc}")
                        nc.sync.dma_start(out=t, in_=abspos[b, kc * 128:(kc + 1) * 128, :])
                        tab.append(t)
                    tmk = a2.tile([128, JC], F32, tag="mk")
                    nc.sync.dma_start(out=tmk, in_=maskT[b])
                    tnm = a2.tile([1, 1], F32, tag="nm")
                    nc.sync.dma_start(out=tnm, in_=norm[b])
                    trn1 = a2.tile([1, 1], F32, tag="rn1")
                    nc.vector.reciprocal(trn1, tnm)
                    trn = a2.tile([128, 1], F32, tag="rn")
                    nc.gpsimd.partition_broadcast(trn, trn1[0:1, :])

                    # ---- x0 = x + vres*xorg ; x1 = x + qko*xorg + qkp*abspos ----
                    tx0, tx1 = [], []
                    for kc in range(KC):
                        t0 = a2.tile([128, L], F32R, tag=f"x0{kc}")
                        nc.vector.scalar_tensor_tensor(
                            t0, txo[kc], tvres[:, kc:kc + 1], tx[kc].bitcast(F32),
                            OP.mult, OP.add)
                        tx0.append(t0)
                        tt = a1.tile([128, L], F32, tag=f"t{kc}")
                        nc.vector.scalar_tensor_tensor(
                            tt, txo[kc], tqko[:, kc:kc + 1], tx[kc].bitcast(F32),
                            OP.mult, OP.add)
                        t1 = a2.tile([128, L], F32R, tag=f"x1{kc}")
                        nc.vector.scalar_tensor_tensor(
                            t1, tab[kc], tqkp[:, kc:kc + 1], tt, OP.mult, OP.add)
                        tx1.append(t1)

                    # ---- gate = gate_w @ x + gate_b ; gateT via identity matmul ----
                    g_ps = psb.tile([NUMHEAD, L], F32, tag="big")
                    for kc in range(KC):
                        nc.tensor.matmul(g_ps, wg[kc], tx[kc],
                                         start=(kc == 0), stop=(kc == KC - 1))
                    tgate = a2.tile([NUMHEAD, L], F32R, tag="gate")
                    nc.any.tensor_scalar(tgate, g_ps, tgb[:, 0:1], None, OP.add)
                    tgm = []
                    for jc in range(JC):
                        gt_ps = psb.tile([128, NUMHEAD], F32, tag="big")
                        nc.tensor.matmul(gt_ps, tgate[:, jc * 128:(jc + 1) * 128], ti8,
                                         start=True, stop=True)
                        gm = a2.tile([128, NUMHEAD], F32, tag=f"gm{jc}")
                        nc.any.tensor_scalar(gm, gt_ps, tmk[:, jc:jc + 1],
                                             trn[:, 0:1], OP.add, OP.mult)
                        tgm.append(gm)

                    # ---- q/k projections -> [chid_chunk][128, L] in fp32r ----
                    def proj(ws, rhs_tiles, tagpfx):
                        outs = []
                        for o in range(OC):
                            ps = psb.tile([128, L], F32, tag="big")
                            for kc in range(KC):
                                nc.tensor.matmul(
                                    ps, ws[kc][:, o * 128:(o + 1) * 128], rhs_tiles[kc],
                                    start=(kc == 0), stop=(kc == KC - 1))
                            t = a2.tile([128, L], F32R, tag=f"{tagpfx}{o}")
                            nc.any.tensor_copy(t, ps)
                            outs.append(t)
                        return outs

                    tq = proj(wq, tx1, "q")
                    tk = proj(wk, tx1, "k")

                    # ---- vT = (x0^T @ v_wT): [l_chunk][128, CHID] fp32r ----
                    tvt = []
                    for lc in range(JC):
                        ps = psb.tile([128, CHID], F32, tag="big")
                        for kc in range(KC):
                            nc.tensor.matmul(
                                ps, tx0[kc][:, lc * 128:(lc + 1) * 128], wv[kc],
                                start=(kc == 0), stop=(kc == KC - 1))
                        t = a2.tile([128, CHID], F32R, tag=f"vt{lc}")
                        nc.any.tensor_copy(t, ps)
                        tvt.append(t)

                    # ---- attention per head ----
                    dn_ps = psd.tile([NUMHEAD, L], F32, tag="dn")
                    toU = []
                    for h in range(NUMHEAD):
                        hp, off = h // 2, 64 * (h % 2)
                        tP = []
                        for jc in range(JC):
                            s_ps = pss.tile([128, L], F32, tag="s")
                            nc.tensor.matmul(
                                s_ps,
                                tk[hp][off:off + 64, jc * 128:(jc + 1) * 128],
                                tq[hp][off:off + 64, :],
                                start=True, stop=False, skip_group_check=True)
                            nc.tensor.matmul(s_ps, ti128, trpt[jc],
                                             start=False, stop=True,
                                             skip_group_check=True)
                            P = ppool.tile([128, L], F32R, tag="P")
                            nc.scalar.activation(P, s_ps, AF.Exp,
                                                 bias=tgm[jc][:, h:h + 1],
                                                 scale=trn[:, 0:1])
                            tP.append(P)
                        o_ps = pso.tile([64, L], F32, tag="o")
                        for jc in range(JC):
                            nc.tensor.matmul(o_ps, tvt[jc][:, 64 * h:64 * h + 64],
                                             tP[jc], start=(jc == 0),
                                             stop=(jc == JC - 1),
                                             skip_group_check=True)
                        for jc in range(JC):
                            nc.tensor.matmul(dn_ps, tsel8[:, 8 * h:8 * h + 8], tP[jc],
                                             start=(h == 0 and jc == 0),
                                             stop=(h == NUMHEAD - 1 and jc == JC - 1),
                                             skip_group_check=True)
                        oU = a1.tile([64, L], F32, tag=f"oU{h}")
                        nc.any.tensor_copy(oU, o_ps)
                        toU.append(oU)

                    # ---- softmax denominators -> reciprocal -> broadcast ----
                    tdnc = a2.tile([NUMHEAD, L], F32, tag="dnc")
                    nc.any.tensor_copy(tdnc, dn_ps)
                    trcp = a2.tile([NUMHEAD, L], F32, tag="rcp")
                    nc.vector.reciprocal_approx_fast(trcp, tdnc)
                    trcpr = a2.tile([NUMHEAD, L], F32R, tag="rcpr")
                    nc.any.tensor_copy(trcpr, trcp)

                    tatt = [a2.tile([128, L], F32R, tag=f"att{o}", name=f"att{o}") for o in range(OC)]
                    for h in range(NUMHEAD):
                        hp, off = h // 2, 64 * (h % 2)
                        rb_ps = psr.tile([64, L], F32, tag="rb")
                        nc.tensor.matmul(rb_ps, tbsel[:, 64 * h:64 * h + 64], trcpr,
                                         start=True, stop=True)
                        nc.vector.tensor_tensor(out=tatt[hp][off:off + 64, :],
                                                in0=toU[h], in1=rb_ps, op=OP.mult)

                    # ---- dense projection + bias ----
                    for o in range(OC):
                        d_ps = psb.tile([128, L], F32, tag="big")
                        for kc in range(KC):
                            nc.tensor.matmul(
                                d_ps, wd[kc][:, o * 128:(o + 1) * 128], tatt[kc],
                                start=(kc == 0), stop=(kc == KC - 1))
                        to = a2.tile([128, L], F32, tag=f"out{o}")
                        nc.any.tensor_scalar(to, d_ps, tdb[:, o:o + 1], None, OP.add)
                        nc.sync.dma_start(out=out[b, o * 128:(o + 1) * 128, :], in_=to)

    nc.compile()
    return nc


_CACHE = {}


def _get_nc(iters=1):
    if iters not in _CACHE:
        _CACHE[iters] = build_nc(iters)
    return _CACHE[iters]


def _host_prep(inputs):
    f32 = lambda a: np.ascontiguousarray(np.asarray(a), dtype=np.float32)
    x, xorg, abspos = f32(inputs["x"]), f32(inputs["xorg"]), f32(inputs["abspos"])
    mask, norm = f32(inputs["mask"]), f32(inputs["norm"])
    relpos = f32(inputs["relpos"])

    def expand_res(r):
        # res[g] applies to channels [8g, 8g+8); channel c -> partition c%128, chunk c//128
        e = np.repeat(f32(r).reshape(-1), DIMGROUP)          # [512]
        return np.ascontiguousarray(e.reshape(KC, 128).T)    # [128, KC]

    shared = {
        "vres": expand_res(inputs["vorg_res"]),
        "qko": expand_res(inputs["qkorg_res"]),
        "qkp": expand_res(inputs["qkpos_res"]),
        "relpos": np.concatenate([relpos, relpos[-1:]]),
        "gate_wT": np.ascontiguousarray(f32(inputs["gate_w"]).T),
        "gate_b": f32(inputs["gate_b"]).reshape(NUMHEAD, 1),
        "q_wT": np.ascontiguousarray(f32(inputs["q_w"]).T),
        "k_wT": np.ascontiguousarray(f32(inputs["k_w"]).T),
        "v_wT": np.ascontiguousarray(f32(inputs["v_w"]).T),
        "dense_wT": np.ascontiguousarray(f32(inputs["dense_w"]).T),
        "dense_b": np.ascontiguousarray(f32(inputs["dense_b"]).reshape(OC, 128).T),
        "i128": np.eye(128, dtype=np.float32),
        "i8": np.eye(NUMHEAD, dtype=np.float32),
    }
    sel8 = np.zeros((128, NUMHEAD * NUMHEAD), np.float32)
    for h in range(NUMHEAD):
        sel8[:, 8 * h + h] = 1.0
    shared["sel8"] = sel8
    bsel = np.zeros((NUMHEAD, NUMHEAD * DIMHEAD), np.float32)
    for h in range(NUMHEAD):
        bsel[h, 64 * h:64 * h + 64] = 1.0
    shared["bsel"] = bsel

    maskT = np.ascontiguousarray(
        mask.reshape(N, JC, 128).transpose(0, 2, 1))         # [N, 128, JC]
    in_maps = []
    for c in range(NCORES):
        sl = slice(BPC * c, BPC * (c + 1))
        m = dict(shared)
        m["x"] = x[sl]
        m["xorg"] = xorg[sl]
        m["abspos"] = abspos[sl]
        m["maskT"] = maskT[sl]
        m["norm"] = norm[sl].reshape(BPC, 1)
        in_maps.append(m)
    return in_maps


def run_on_hw(inputs, iters=1):
    nc = _get_nc(iters)
    in_maps = _host_prep(inputs)
    res = run_bass_kernel_spmd(nc, in_maps, list(range(NCORES)))
    return np.concatenate([res.results[c]["out"] for c in range(NCORES)], axis=0)


def kernel(**inputs) -> np.ndarray:
    return run_on_hw(inputs, iters=1)


# revision 31
# speedup vs baseline: 82.5307x; 82.5307x over previous
import sys
sys.path.insert(0, "/opt/trn_rl_repo")
import numpy as np
import concourse.bass as bass
import concourse.bacc as bacc
import concourse.tile as tile
from concourse import mybir
from concourse.bass_utils import run_bass_kernel_spmd
from concourse import bass_isa

F32 = mybir.dt.float32
F32R = mybir.dt.float32r
BF16 = mybir.dt.bfloat16
AF = mybir.ActivationFunctionType
OP = mybir.AluOpType

N, CIO, L = 16, 512, 384
DIMHEAD, NUMHEAD, MAXEMBED, DIMGROUP = 64, 8, 384, 8
CHID = DIMHEAD * NUMHEAD
NCORES = 8
BPC = N // NCORES      # batches per core
KC = CIO // 128        # contraction chunks
OC = CHID // 128       # output-channel chunks
JC = L // 128          # key-position chunks

# constant-blob column layout (128-partition blob, fp32r-typed)
CB_I128 = 0            # [128,128] identity
CB_SEL8 = 128          # [128,64] head-selector columns
CB_VRES = 192          # [128,4]
CB_QKO = 196           # [128,4]
CB_QKP = 200           # [128,4]
CB_DB = 204            # [128,4] dense bias
CB_ONES = 208          # [128,64] all-ones
CB_COLS = 272
# 8-partition blob
B8_BSEL = 0            # [8,512]
B8_I8 = 512            # [8,8]
B8_GB = 520            # [8,1] gate bias
B8_COLS = 521


def build_nc(iters=1):
    nc = bacc.Bacc("TRN2", target_bir_lowering=False, debug=False)

    def inp(name, shape, dt):
        return nc.dram_tensor(name, shape, dt, kind="ExternalInput").ap()

    x = inp("x", [BPC, CIO, L], F32R)
    xorg = inp("xorg", [BPC, CIO, L], F32)
    abspos = inp("abspos", [BPC, CIO, L], F32)
    mblob = inp("mblob", [BPC, 128, 4], F32)     # [maskT(3) | norm(1)]
    rptin = inp("rptin", [128, JC * L], BF16)
    i128b = inp("i128b", [128, 128], BF16)
    gate_wT = inp("gate_wT", [CIO, NUMHEAD], F32R)
    q_wT = inp("q_wT", [CIO, CHID], F32R)
    k_wT = inp("k_wT", [CIO, CHID], F32R)
    v_wT = inp("v_wT", [CIO, CHID], F32R)
    dense_wT = inp("dense_wT", [CHID, CIO], F32R)
    cblob = inp("cblob", [128, CB_COLS], F32R)
    blob8 = inp("blob8", [NUMHEAD, B8_COLS], F32R)
    out = nc.dram_tensor("out", [BPC, CIO, L], F32, kind="ExternalOutput").ap()

    def chunked_src(t, b):
        # [CIO, L] DRAM slice viewed as [p(128), kc(4), l(384)]
        return bass.AP(tensor=t.tensor, offset=b * CIO * L,
                       ap=[[L, 128], [128 * L, KC], [1, L]])

    def wsrc(t):
        # [CIO, CHID] viewed as [p(128), kc(4), c(CHID)]
        return bass.AP(tensor=t.tensor, offset=0,
                       ap=[[CHID, 128], [128 * CHID, KC], [1, CHID]])

    with tile.TileContext(nc) as tc:
        with tc.tile_pool(name="wts", bufs=1) as wp, \
             tc.tile_pool(name="a2", bufs=2) as a2, \
             tc.tile_pool(name="a1", bufs=1) as a1, \
             tc.tile_pool(name="pp", bufs=9) as ppool, \
             tc.tile_pool(name="ps_big", bufs=3, space="PSUM") as psb, \
             tc.tile_pool(name="ps_s", bufs=3, space="PSUM") as pss, \
             tc.tile_pool(name="ps_o", bufs=2, space="PSUM") as pso:

            first = [True]
            weights = {}

            def emit_weight_dmas():
                w = weights
                t = wp.tile([128, CB_COLS], F32R, tag="cb", name="cb")
                nc.scalar.dma_start(out=t, in_=cblob)
                w["cb"] = t
                t = wp.tile([NUMHEAD, B8_COLS], F32R, tag="b8", name="b8")
                nc.scalar.dma_start(out=t, in_=blob8)
                w["b8"] = t
                w["mb"] = []
                for b in range(BPC):
                    t = wp.tile([128, 4], F32, tag=f"mb{b}", name=f"mb{b}")
                    nc.scalar.dma_start(out=t, in_=mblob[b])
                    w["mb"].append(t)
                t = wp.tile([128, KC * NUMHEAD], F32R, tag="wg", name="wg")
                nc.scalar.dma_start(
                    out=t, in_=bass.AP(tensor=gate_wT.tensor, offset=0,
                                       ap=[[NUMHEAD, 128], [128 * NUMHEAD, KC],
                                           [1, NUMHEAD]]))
                w["wg"] = t
                for nm, src in (("wq", q_wT), ("wk", k_wT)):
                    t = wp.tile([128, KC * CHID], F32R, tag=nm, name=nm)
                    nc.scalar.dma_start(out=t, in_=wsrc(src))
                    w[nm] = t
                t = wp.tile([128, 128], BF16, tag="i128b", name="i128b")
                nc.scalar.dma_start(out=t, in_=i128b)
                w["i128b"] = t
                t = wp.tile([128, JC * L], BF16, tag="rpt", name="rpt")
                nc.scalar.dma_start(out=t, in_=rptin)
                w["rpt"] = [t[:, jc * L:(jc + 1) * L] for jc in range(JC)]
                for nm, src in (("wv", v_wT), ("wd", dense_wT)):
                    t = wp.tile([128, KC * CHID], F32R, tag=nm, name=nm)
                    nc.scalar.dma_start(out=t, in_=wsrc(src))
                    w[nm] = t

            for it in range(iters):
                binp = []

                def emit_inp(b, ab_engine=None):
                    txa = a1.tile([128, KC * L], F32R, tag=f"xa{b}", name=f"xa{b}")
                    nc.sync.dma_start(out=txa, in_=chunked_src(x, b))
                    txo = a1.tile([128, KC * L], F32, tag=f"xo{b}", name=f"xo{b}")
                    nc.sync.dma_start(out=txo, in_=chunked_src(xorg, b))
                    tab = a1.tile([128, KC * L], F32, tag=f"ab{b}", name=f"ab{b}")
                    (ab_engine or nc.gpsimd).dma_start(out=tab,
                                                       in_=chunked_src(abspos, b))
                    binp.append((txa, txo, tab))

                def wdma(nm, shape, dt, srcap):
                    t = wp.tile([128, shape], dt, tag=nm, name=nm)
                    nc.scalar.dma_start(out=t, in_=srcap)
                    weights[nm] = t
                    return t

                if first[0]:
                    w = weights
                    emit_inp(0)
                    t = wp.tile([NUMHEAD, B8_COLS], F32R, tag="b8", name="b8")
                    nc.scalar.dma_start(out=t, in_=blob8)
                    w["b8"] = t
                    wdma("wg", KC * NUMHEAD, F32R,
                         bass.AP(tensor=gate_wT.tensor, offset=0,
                                 ap=[[NUMHEAD, 128], [128 * NUMHEAD, KC],
                                     [1, NUMHEAD]]))
                    w["mb"] = []
                    for b in range(BPC):
                        t = wp.tile([128, 4], F32, tag=f"mb{b}", name=f"mb{b}")
                        nc.scalar.dma_start(out=t, in_=mblob[b])
                        w["mb"].append(t)
                    wdma("cb", CB_COLS, F32R, cblob)
                    wdma("wq", KC * CHID, F32R, wsrc(q_wT))
                    wdma("wk", KC * CHID, F32R, wsrc(k_wT))
                    wdma("i128b", 128, BF16, i128b)
                    rt = wdma("rptw", JC * L, BF16, rptin)
                    w["rpt"] = [rt[:, jc * L:(jc + 1) * L] for jc in range(JC)]
                    wdma("wv", KC * CHID, F32R, wsrc(v_wT))
                    wdma("wd", KC * CHID, F32R, wsrc(dense_wT))
                    emit_inp(1, ab_engine=nc.sync)
                    first[0] = False
                else:
                    emit_inp(0)
                    emit_inp(1)
                w = weights
                cb, b8 = w["cb"], w["b8"]

                for b in range(BPC):
                    txa, txo, tab = binp[b]
                    xs = lambda t, kc: t[:, kc * L:(kc + 1) * L]

                    rn = a2.tile([128, 1], F32, tag="rn", name="rn")
                    nc.vector.reciprocal(rn, w["mb"][b][:, 3:4])

                    # ---- x0 = x + vres*xorg ; x1 = x + qko*xorg + qkp*abspos ----
                    tx0 = a1.tile([128, KC * L], F32R, tag="x0", name="x0")
                    tx1 = a1.tile([128, KC * L], F32R, tag="x1", name="x1")
                    ttm = a1.tile([128, KC * L], F32, tag="tt", name="tt")
                    tmv = a1.tile([128, KC * L], F32, tag="tmv", name="tmv")
                    tmv2 = a1.tile([128, KC * L], F32, tag="tmv2", name="tmv2")

                    def bcast1(col):
                        sv = cb[:, col:col + 1].bitcast(F32)
                        return bass.AP(tensor=sv.tensor, offset=sv.offset,
                                       ap=[sv.ap[0], [0, L]])

                    for kc in range(KC):
                        nc.gpsimd.tensor_tensor(out=xs(tmv, kc), in0=xs(txo, kc),
                                                in1=bcast1(CB_VRES + kc), op=OP.mult)
                        nc.gpsimd.tensor_tensor(out=xs(tx0, kc), in0=xs(tmv, kc),
                                                in1=xs(txa, kc).bitcast(F32),
                                                op=OP.add)
                        nc.gpsimd.tensor_tensor(out=xs(tmv2, kc), in0=xs(txo, kc),
                                                in1=bcast1(CB_QKO + kc), op=OP.mult)
                        nc.gpsimd.tensor_tensor(out=xs(ttm, kc), in0=xs(tmv2, kc),
                                                in1=xs(txa, kc).bitcast(F32),
                                                op=OP.add)
                        nc.vector.scalar_tensor_tensor(
                            xs(tx1, kc), xs(tab, kc),
                            cb[:, CB_QKP + kc:CB_QKP + kc + 1].bitcast(F32),
                            xs(ttm, kc), OP.mult, OP.add)

                    # ---- gate projection (PE) ----
                    g_ps = psb.tile([NUMHEAD, L], F32, tag="big", name="g_ps")
                    for kc in range(KC):
                        nc.tensor.matmul(
                            g_ps, w["wg"][:, kc * NUMHEAD:(kc + 1) * NUMHEAD],
                            xs(txa, kc), start=(kc == 0), stop=(kc == KC - 1))
                    tgate = a2.tile([NUMHEAD, L], F32R, tag="gate", name="gate")
                    nc.scalar.activation(tgate, g_ps, AF.Identity,
                                         bias=b8[:, B8_GB:B8_GB + 1].bitcast(F32))

                    # ---- q/k projections (PE busy while gate transposes) ----
                    def proj(wall, rhs_all, nm):
                        dst = a2.tile([128, OC * L], F32R, tag=nm, name=nm)
                        for o in range(OC):
                            ps = psb.tile([128, L], F32, tag="big",
                                          name=f"{nm}p{o}")
                            for kc in range(KC):
                                nc.tensor.matmul(
                                    ps,
                                    wall[:, kc * CHID + o * 128:
                                         kc * CHID + o * 128 + 128],
                                    xs(rhs_all, kc),
                                    start=(kc == 0), stop=(kc == KC - 1))
                            nc.vector.tensor_copy(dst[:, o * L:(o + 1) * L], ps)
                        return dst

                    tq = proj(w["wq"], tx1, "q")

                    # gate transpose + gm (interleaved with q/k on other engines)
                    tgm = []
                    for jc in range(JC):
                        gt_ps = psb.tile([128, NUMHEAD], F32, tag="big",
                                         name=f"gt{jc}")
                        nc.tensor.matmul(gt_ps,
                                         tgate[:, jc * 128:(jc + 1) * 128],
                                         b8[:, B8_I8:B8_I8 + NUMHEAD],
                                         start=True, stop=True)
                        gm = a2.tile([128, NUMHEAD], F32, tag=f"gm{jc}",
                                     name=f"gm{jc}")
                        nc.vector.tensor_scalar(gm, gt_ps,
                                                w["mb"][b][:, jc:jc + 1],
                                                rn[:, 0:1], OP.add, OP.mult)
                        tgm.append(gm)

                    tk = proj(w["wk"], tx1, "k")

                    # ---- vT = x0^T @ v_wT ; per-head 65-col blocks, col 64 = ones ----
                    VW = DIMHEAD + 1
                    tvt = a2.tile([128, JC * NUMHEAD * VW], F32R, tag="vt", name="vt")

                    for lc in range(JC):
                        ps = psb.tile([128, CHID], F32, tag="big", name=f"vtp{lc}")
                        for kc in range(KC):
                            nc.tensor.matmul(
                                ps, tx0[:, kc * L + lc * 128:kc * L + lc * 128 + 128],
                                w["wv"][:, kc * CHID:(kc + 1) * CHID],
                                start=(kc == 0), stop=(kc == KC - 1))
                        blk = tvt[:, lc * NUMHEAD * VW:(lc + 1) * NUMHEAD * VW]
                        blk = blk.rearrange("p (h c) -> p h c", c=VW)
                        nc.vector.tensor_copy(blk[:, :, 0:DIMHEAD], ps)
                        nc.vector.tensor_copy(blk[:, :, DIMHEAD:VW],
                                              cb[:, CB_ONES:CB_ONES + NUMHEAD])

                    # ---- attention (emission software-pipelined over heads) ----
                    tP_heads = {}
                    tatt = a1.tile([128, OC * L], F32R, tag="att", name="att")

                    def emit_qk(h):
                        hp, off = h // 2, 64 * (h % 2)
                        tP = []
                        for jc in range(JC):
                            s_ps = pss.tile([128, L], F32, tag="s", name=f"s{h}{jc}")
                            nc.tensor.matmul(
                                s_ps,
                                tk[off:off + 64,
                                   hp * L + jc * 128:hp * L + jc * 128 + 128],
                                tq[off:off + 64, hp * L:(hp + 1) * L],
                                start=True, stop=False, skip_group_check=True)
                            nc.tensor.matmul(s_ps, w["i128b"],
                                             w["rpt"][jc], start=False, stop=True,
                                             skip_group_check=True)
                            P = ppool.tile([128, L], F32R, tag="P", name=f"P{h}{jc}")
                            nc.scalar.activation(P, s_ps, AF.Exp,
                                                 bias=tgm[jc][:, h:h + 1],
                                                 scale=rn[:, 0:1])
                            tP.append(P)
                        tP_heads[h] = tP

                    def emit_av(h):
                        tP = tP_heads.pop(h)
                        hp, off = h // 2, 64 * (h % 2)
                        o_ps = pso.tile([VW, L], F32, tag="o", name=f"o{h}")
                        for jc in range(JC):
                            nc.tensor.matmul(
                                o_ps,
                                tvt[:, jc * NUMHEAD * VW + VW * h:
                                    jc * NUMHEAD * VW + VW * h + VW],
                                tP[jc], start=(jc == 0), stop=(jc == JC - 1),
                                skip_group_check=True)
                        oU = a1.tile([VW, L], F32, tag=f"oU{h}", name=f"oU{h}")
                        nc.vector.tensor_copy(oU, o_ps)
                        rcp65 = a2.tile([VW, L], F32, tag="rcp65", name=f"rcp{h}")
                        nc.vector.reciprocal(rcp65[64:65, :], oU[64:65, :])
                        rcpr65 = a2.tile([VW, L], F32R, tag="rcpr65", name=f"rr{h}")
                        nc.scalar.copy(rcpr65[64:65, :], rcp65[64:65, :])
                        rb_ps = psb.tile([64, L], F32, tag="big", name=f"rb{h}")
                        nc.tensor.matmul(rb_ps, cb[64:65, CB_ONES:CB_ONES + 64],
                                         rcpr65[64:65, :], start=True, stop=True)
                        nc.vector.tensor_tensor(
                            out=tatt[off:off + 64, hp * L:(hp + 1) * L],
                            in0=oU[0:64, :], in1=rb_ps, op=OP.mult)

                    emit_qk(0)
                    emit_qk(1)
                    tout = a2.tile([128, OC * L], F32, tag="outt", name="outt")
                    d_ps = {}

                    def emit_dense_partial(o, kcs, stop):
                        if o not in d_ps:
                            d_ps[o] = psb.tile([128, L], F32, tag="big",
                                               name=f"dp{o}")
                        for kc in kcs:
                            nc.tensor.matmul(
                                d_ps[o],
                                w["wd"][:, kc * CIO + o * 128:kc * CIO + o * 128 + 128],
                                tatt[:, kc * L:(kc + 1) * L],
                                start=(kc == 0), stop=(stop and kc == kcs[-1]),
                                skip_group_check=True)
                        if stop:
                            nc.scalar.activation(
                                tout[:, o * L:(o + 1) * L], d_ps.pop(o), AF.Identity,
                                bias=cb[:, CB_DB + o:CB_DB + o + 1].bitcast(F32))

                    for h in range(NUMHEAD):
                        if h + 2 < NUMHEAD:
                            emit_qk(h + 2)
                        emit_av(h)
                        if h == NUMHEAD - 2:
                            for o in (0, 1):
                                emit_dense_partial(o, [0, 1, 2], stop=False)
                    for o in (0, 1):
                        emit_dense_partial(o, [3], stop=True)
                    for o in (2, 3):
                        emit_dense_partial(o, [0, 1, 2, 3], stop=True)
                    dst = bass.AP(tensor=out.tensor, offset=b * CIO * L,
                                  ap=[[L, 128], [128 * L, KC], [1, L]])
                    nc.sync.dma_start(out=dst, in_=tout)

    nc.compile()
    return nc


_CACHE = {}


def _get_nc(iters=1):
    if iters not in _CACHE:
        _CACHE[iters] = build_nc(iters)
    return _CACHE[iters]


def _bf16(a):
    import ml_dtypes
    return np.asarray(a, dtype=ml_dtypes.bfloat16)


def _make_rpt(relpos):
    # rpt[p, jc*L + i] = relpos[clip(384 + 128*jc + p - i, 0, 766)]
    j = np.arange(L)[:, None]
    i = np.arange(L)[None, :]
    idx = np.clip(MAXEMBED + j - i, 0, 2 * MAXEMBED - 2)
    rp = relpos[idx]                                  # [j, i]
    return np.ascontiguousarray(
        rp.reshape(JC, 128, L).transpose(1, 0, 2).reshape(128, JC * L))


def _host_prep(inputs):
    f32 = lambda a: np.ascontiguousarray(np.asarray(a), dtype=np.float32)
    x, xorg, abspos = f32(inputs["x"]), f32(inputs["xorg"]), f32(inputs["abspos"])
    mask, norm = f32(inputs["mask"]), f32(inputs["norm"])
    relpos = f32(inputs["relpos"])

    def expand_res(r):
        e = np.repeat(f32(r).reshape(-1), DIMGROUP)          # [512]
        return np.ascontiguousarray(e.reshape(KC, 128).T)    # [128, KC]

    cblob = np.zeros((128, CB_COLS), np.float32)
    cblob[:, CB_I128:CB_I128 + 128] = np.eye(128, dtype=np.float32)
    for h in range(NUMHEAD):
        cblob[:, CB_SEL8 + 8 * h + (h % 2)] = 1.0
    cblob[:, CB_VRES:CB_VRES + KC] = expand_res(inputs["vorg_res"])
    cblob[:, CB_QKO:CB_QKO + KC] = expand_res(inputs["qkorg_res"])
    cblob[:, CB_QKP:CB_QKP + KC] = expand_res(inputs["qkpos_res"])
    cblob[:, CB_DB:CB_DB + KC] = np.ascontiguousarray(
        f32(inputs["dense_b"]).reshape(OC, 128).T)
    cblob[:, CB_ONES:CB_ONES + 64] = 1.0

    blob8 = np.zeros((NUMHEAD, B8_COLS), np.float32)
    for h in range(NUMHEAD):
        blob8[h % 2, B8_BSEL + 64 * h:B8_BSEL + 64 * h + 64] = 1.0
    blob8[:, B8_I8:B8_I8 + NUMHEAD] = np.eye(NUMHEAD, dtype=np.float32)
    blob8[:, B8_GB] = f32(inputs["gate_b"])

    shared = {
        "rptin": _bf16(_make_rpt(relpos)),
        "i128b": _bf16(np.eye(128, dtype=np.float32)),
        "gate_wT": np.ascontiguousarray(f32(inputs["gate_w"]).T),
        "q_wT": np.ascontiguousarray(f32(inputs["q_w"]).T),
        "k_wT": np.ascontiguousarray(f32(inputs["k_w"]).T),
        "v_wT": np.ascontiguousarray(f32(inputs["v_w"]).T),
        "dense_wT": np.ascontiguousarray(f32(inputs["dense_w"]).T),
        "cblob": cblob,
        "blob8": blob8,
    }
    # mblob: [N, 128, 4] = [maskT(3) | norm(1)]
    mblob = np.zeros((N, 128, 4), np.float32)
    mblob[:, :, 0:3] = mask.reshape(N, JC, 128).transpose(0, 2, 1)
    mblob[:, :, 3] = norm[:, None]
    in_maps = []
    for c in range(NCORES):
        sl = slice(BPC * c, BPC * (c + 1))
        m = dict(shared)
        m["x"] = x[sl]
        m["xorg"] = xorg[sl]
        m["abspos"] = abspos[sl]
        m["mblob"] = mblob[sl]
        in_maps.append(m)
    return in_maps


def run_on_hw(inputs, iters=1):
    nc = _get_nc(iters)
    in_maps = _host_prep(inputs)
    res = run_bass_kernel_spmd(nc, in_maps, list(range(NCORES)))
    return np.concatenate([res.results[c]["out"] for c in range(NCORES)], axis=0)


def kernel(**inputs) -> np.ndarray:
    return run_on_hw(inputs, iters=1)
